# revision 15
# baseline (speedup 1.0000x reference)
"""FFM pairwise-interaction kernel for Trainium2 (8 NeuronCores, batch-sharded).

out[b, p*64+e] = x[b, i, e] * x[b, j, e] * fe[i, j, e] * fe[j, i, e]
for the p-th pair (i, j), i < j, in row-major triu order.

Per-core strategy (batch shard of 512 rows, 4 tiles of 128 on partitions).
All arithmetic fp32 (bit-accurate vs the fp32 reference):

  inter_c[e, p] = fe[i,j,e]*fe[j,i,e], e-major [64, 780], via tiny GpSimd
      TTs on a PE-transposed copy of fe.
  Per chunk (group of whole pair-blocks, <= 4160 columns):
      W [64, cols]  = block-diagonal fp32 weights (GpSimd affine_select of
                      a broadcast view of inter_c; keep where e == part).
      rep [128, cols] = ones64.T @ W on TensorE (exact: one nonzero per
                      column) -> PSUM pieces -> ScalarE copy -> SBUF.
      Per batch tile t:
          pass 1 (VectorE): ob = x_i(step-0 bcast) * x_suffix  per block
          pass 2 (VectorE): ob *= rep   (single big op per chunk)
          DMA ob -> HBM.
  VectorE does the two irreducible fp32 elementwise passes (~416us floor);
  everything else rides idle engines.
"""

import numpy as np

import concourse.bass as bass
import concourse.mybir as mybir
import concourse.tile as tile
from concourse import bacc, bass_utils

F32 = mybir.dt.float32

N_CORES = 8
B_FULL = 4096
F = 40
E = 64
B = B_FULL // N_CORES          # 512 rows per core
P = 128                        # SBUF partitions
N_TILES = B // P               # 4
PAIRS = F * (F - 1) // 2       # 780
OUT_COLS = PAIRS * E           # 49920

BLOCK_OFF = []
_off = 0
for _i in range(F - 1):
    BLOCK_OFF.append(_off)
    _off += (F - 1 - _i) * E
assert _off == OUT_COLS

CHUNK_CAP = 4160               # columns per streamed chunk (65 pairs)
REP_PIECE = 1536               # PSUM piece for the ones-matmul (3 banks)


def _chunks():
    chunks = []
    cur_blocks, cur_cols = [], 0
    for i in range(F - 1):
        c = (F - 1 - i) * E
        if cur_blocks and cur_cols + c > CHUNK_CAP:
            chunks.append((BLOCK_OFF[cur_blocks[0]], cur_cols, cur_blocks))
            cur_blocks, cur_cols = [], 0
        cur_blocks.append(i)
        cur_cols += c
    chunks.append((BLOCK_OFF[cur_blocks[0]], cur_cols, cur_blocks))
    return chunks


CHUNKS = _chunks()


def build_nc() -> bass.Bass:
    nc = bacc.Bacc(
        "TRN2",
        target_bir_lowering=False,
        debug=False,
        enable_asserts=False,
        num_devices=N_CORES,
    )
    x = nc.dram_tensor("x", [B, F * E], F32, kind="ExternalInput")
    fe = nc.dram_tensor("feat_embedding", [F * F, E], F32, kind="ExternalInput")
    out = nc.dram_tensor("out", [B, OUT_COLS], F32, kind="ExternalOutput")

    ident_np = np.eye(P, dtype=np.float32)

    with tile.TileContext(nc) as tc:
        ident_dram = nc.inline_tensor(ident_np, name="ident")
        with (
            tc.tile_pool(name="xp", bufs=1) as xp,
            tc.tile_pool(name="cst", bufs=1) as cst,
            tc.tile_pool(name="fp", bufs=2) as fp,
            tc.tile_pool(name="wp", bufs=2) as wp,
            tc.tile_pool(name="repp", bufs=2) as repp,
            tc.tile_pool(name="obp", bufs=5) as obp,
            tc.tile_pool(name="psp", bufs=2, space="PSUM") as psp,
            tc.tile_pool(name="tp", bufs=1, space="PSUM") as tp,
        ):
            ident = cst.tile([P, P], F32, tag="ident")
            nc.sync.dma_start(out=ident[:], in_=ident_dram[:, :])
            ones1 = cst.tile([1, P], F32, tag="ones1")
            nc.vector.memset(ones1[:], 1.0)

            x_sb = []
            for t in range(N_TILES):
                xt = xp.tile([P, F * E], F32, tag=f"x{t}")
                nc.sync.dma_start(out=xt[:], in_=x[t * P : (t + 1) * P, :])
                x_sb.append(xt)

            # ---- icT: inter in pair-major [pair, e] pieces of 128 pairs ----
            # triu/tril loaded straight from HBM fe (per-block contiguous /
            # strided row segments), multiplied on GpSimd.
            n_ipieces = (PAIRS + P - 1) // P  # 7
            icT = cst.tile([P, n_ipieces * E], F32, tag="icT")
            for k in range(n_ipieces):
                sz = min(P, PAIRS - k * P)
                tu = fp.tile([P, E], F32, tag="tu")
                tl = fp.tile([P, E], F32, tag="tl")
                # walk blocks overlapping pairs [k*128, k*128+sz)
                q = k * P
                while q < k * P + sz:
                    i = 0
                    while not (BLOCK_OFF[i] // E <= q < BLOCK_OFF[i] // E + F - 1 - i):
                        i += 1
                    qoff = BLOCK_OFF[i] // E
                    j0 = i + 1 + (q - qoff)     # first j of this segment
                    take = min(F - j0, k * P + sz - q)
                    r = q - k * P
                    # triu rows fe[i, j0 : j0+take, :] (contiguous rows)
                    nc.sync.dma_start(
                        out=tu[r : r + take, :],
                        in_=fe[i * F + j0 : i * F + j0 + take, :],
                    )
                    # tril rows fe[j, i, :], j = j0.. (stride F rows)
                    nc.scalar.dma_start(
                        out=tl[r : r + take, :],
                        in_=fe[j0 * F + i : (j0 + take) * F : F, :],
                    )
                    q += take
                nc.gpsimd.tensor_mul(
                    out=icT[:sz, k * E : (k + 1) * E],
                    in0=tu[:sz, :],
                    in1=tl[:sz, :],
                )

            # ---- main loop over chunks ----
            for coff, cols, blocks in CHUNKS:
                q0 = coff // E
                nq_c = cols // E
                # flatten chunk's inter rows into [1, cols] on partition 0
                # (1-2 SBUF->SBUF DMAs, crossing icT piece boundaries)
                ifl = wp.tile([1, CHUNK_CAP], F32, tag="ifl")
                qq = q0
                while qq < q0 + nq_c:
                    k = qq // P
                    take = min(q0 + nq_c - qq, (k + 1) * P - qq)
                    nc.scalar.dma_start(
                        out=ifl[0:1, (qq - q0) * E : (qq - q0 + take) * E],
                        in_=icT[qq - k * P : qq - k * P + take, k * E : (k + 1) * E],
                    )
                    qq += take
                # rep [128, cols] = ones1.T @ ifl  (K=1 PE matmuls, exact)
                rep = repp.tile([P, CHUNK_CAP], F32, tag="rep")
                p0 = 0
                while p0 < cols:
                    pc = min(REP_PIECE, cols - p0)
                    pz = psp.tile([P, REP_PIECE], F32, tag="pz")
                    s0 = 0
                    while s0 < pc:
                        sc = min(512, pc - s0)
                        nc.tensor.matmul(
                            pz[:, s0 : s0 + sc],
                            ones1[:],
                            ifl[0:1, p0 + s0 : p0 + s0 + sc],
                            start=True,
                            stop=True,
                        )
                        s0 += sc
                    nc.scalar.copy(rep[:, p0 : p0 + pc], pz[:, :pc])
                    p0 += pc

                for t in range(N_TILES):
                    ob = obp.tile([P, CHUNK_CAP], F32, tag="ob")
                    for b in blocks:
                        nq = F - 1 - b
                        seg = BLOCK_OFF[b] - coff
                        xi = (
                            x_sb[t][:, b * E : (b + 1) * E]
                            .unsqueeze(1)
                            .broadcast_to([P, nq, E])
                        )
                        xj = x_sb[t][:, (b + 1) * E : F * E].rearrange(
                            "p (q e) -> p q e", e=E
                        )
                        o = ob[:, seg : seg + nq * E].rearrange(
                            "p (q e) -> p q e", e=E
                        )
                        nc.vector.tensor_mul(out=o, in0=xi, in1=xj)
                    nc.vector.tensor_mul(
                        out=ob[:, :cols], in0=ob[:, :cols], in1=rep[:, :cols]
                    )
                    nc.sync.dma_start(
                        out=out[t * P : (t + 1) * P, coff : coff + cols],
                        in_=ob[:, :cols],
                    )
    nc.finalize()
    return nc


_NC = None


def _get_nc():
    global _NC
    if _NC is None:
        _NC = build_nc()
    return _NC


def kernel(x: np.ndarray, feat_embedding: np.ndarray, trace: bool = False):
    assert x.shape == (B_FULL, F, E) and feat_embedding.shape == (F, F, E)
    x = np.ascontiguousarray(x, dtype=np.float32).reshape(B_FULL, F * E)
    fe = np.ascontiguousarray(feat_embedding, dtype=np.float32).reshape(F * F, E)
    nc = _get_nc()
    in_maps = [
        {"x": x[c * B : (c + 1) * B], "feat_embedding": fe} for c in range(N_CORES)
    ]
    res = bass_utils.run_bass_kernel_spmd(
        nc, in_maps, core_ids=list(range(N_CORES)), trace=trace
    )
    kernel.last_result = res
    return np.concatenate([r["out"] for r in res.results], axis=0)
